# revision 8
# baseline (speedup 1.0000x reference)
"""CoAttLayer Trainium2 kernel — pure data-parallel over batch on 8 NeuronCores.

Reference computation (per batch element b, T=1024, N=512, D=64, K=80):
  L  = tanh(R @ Wl @ P^T)                    (T, N)
  Hp = tanh(Wp @ P^T + (Wr @ R^T) @ L)       (K, N)
  Hr = tanh(Wr @ R^T + (Wp @ P^T) @ L^T)     (K, T)
  Ap = softmax(whp @ Hp), Ar = softmax(whr @ Hr)
  out[b] = concat(P^T @ Ap, R^T @ Ar)        (2D,)

We reassociate the K-sized einsums into D-sized contractions:
  Hp = [Wp | Wr] @ [P^T ; X]   with X = R^T @ L    (D, N)
  Hr = [Wr | Wp] @ [R^T ; Y]   with Y = P^T @ L^T  (D, T)
so the big L-contractions contract over T resp. N with D=64 outputs, and the
Hp/Hr matmuls become single K=128-contraction matmuls.

Each core processes 8 batch elements entirely in SBUF (L never hits DRAM).
"""

import numpy as np

import concourse.bass as bass
import concourse.bacc as bacc
import concourse.mybir as mybir
import concourse.tile as tile
from concourse.bass_utils import run_bass_kernel_spmd

F32 = mybir.dt.float32
BF16 = mybir.dt.bfloat16
AF = mybir.ActivationFunctionType

B_LOC = 8      # batch elements per core
T, N, D, K = 1024, 512, 64, 80
TI = T // 128  # 8 t-tiles
NI = N // 128  # 4 n-tiles
NCORES = 8


def build_kernel():
    nc = bacc.Bacc("TRN2", debug=False, target_bir_lowering=False)

    rev_e = nc.declare_dram_parameter("review_seq", [B_LOC, T, D], F32, isOutput=False)
    post_e = nc.declare_dram_parameter("post_seq", [B_LOC, N, D], F32, isOutput=False)
    wl_e = nc.declare_dram_parameter("Wl", [D, D], F32, isOutput=False)
    wr_e = nc.declare_dram_parameter("Wr", [K, D], F32, isOutput=False)
    wp_e = nc.declare_dram_parameter("Wp", [K, D], F32, isOutput=False)
    whr_e = nc.declare_dram_parameter("whr", [1, K], F32, isOutput=False)
    whp_e = nc.declare_dram_parameter("whp", [1, K], F32, isOutput=False)
    id_e = nc.declare_dram_parameter("ident", [128, 128], F32, isOutput=False)
    out_e = nc.declare_dram_parameter("out", [B_LOC, 2 * D], F32, isOutput=True)

    with tile.TileContext(nc) as tc:
        _body(nc, tc, rev_e, post_e, wl_e, wr_e, wp_e, whr_e, whp_e, id_e, out_e)

    nc.compile()
    return nc


def _body(nc, tc, rev_e, post_e, wl_e, wr_e, wp_e, whr_e, whp_e, id_e, out_e):
    from contextlib import ExitStack

    ctx = ExitStack()
    cpool = ctx.enter_context(tc.tile_pool(name="const", bufs=1))
    inpool = ctx.enter_context(tc.tile_pool(name="inputs", bufs=1))
    wk = ctx.enter_context(tc.tile_pool(name="work", bufs=2))
    ps_mm = ctx.enter_context(tc.tile_pool(name="ps_mm", bufs=3, space="PSUM"))
    ps_tp = ctx.enter_context(tc.tile_pool(name="ps_tp", bufs=2, space="PSUM"))
    ps_sm = ctx.enter_context(tc.tile_pool(name="ps_sm", bufs=2, space="PSUM"))

    # ---------------- constants / weights prologue ----------------
    ident_f = cpool.tile([128, 128], F32)
    nc.sync.dma_start(out=ident_f[:], in_=id_e.ap())
    ident_b = cpool.tile([128, 128], BF16)
    nc.vector.tensor_copy(ident_b[:], ident_f[:])

    wl_f = cpool.tile([D, D], F32)
    nc.sync.dma_start(out=wl_f[:], in_=wl_e.ap())
    wl_b = cpool.tile([D, D], BF16)
    nc.vector.tensor_copy(wl_b[:], wl_f[:])

    wr_f = cpool.tile([K, D], F32)
    nc.sync.dma_start(out=wr_f[:], in_=wr_e.ap())
    wr_b = cpool.tile([K, D], BF16)
    nc.vector.tensor_copy(wr_b[:], wr_f[:])

    wp_f = cpool.tile([K, D], F32)
    nc.sync.dma_start(out=wp_f[:], in_=wp_e.ap())
    wp_b = cpool.tile([K, D], BF16)
    nc.vector.tensor_copy(wp_b[:], wp_f[:])

    # Transposed weight stacks for the merged Hp/Hr matmuls:
    #   WT_hp = [Wp^T ; Wr^T]  (128, K)  (rows 0:64 contract with P^T, 64:128 with X)
    #   WT_hr = [Wr^T ; Wp^T]  (128, K)
    wpt_ps = ps_sm.tile([D, K], BF16, tag="sm")
    nc.tensor.transpose(wpt_ps[:], wp_b[:], ident_b[0:K, 0:K])
    wrt_ps = ps_sm.tile([D, K], BF16, tag="sm")
    nc.tensor.transpose(wrt_ps[:], wr_b[:], ident_b[0:K, 0:K])
    wt_hp = cpool.tile([128, K], BF16)
    wt_hr = cpool.tile([128, K], BF16)
    nc.vector.tensor_copy(wt_hp[0:D, :], wpt_ps[:])
    nc.vector.tensor_copy(wt_hp[D:128, :], wrt_ps[:])
    nc.vector.tensor_copy(wt_hr[0:D, :], wrt_ps[:])
    nc.vector.tensor_copy(wt_hr[D:128, :], wpt_ps[:])

    # whp/whr as column vectors (K, 1), bf16
    whp_f = cpool.tile([K, 1], F32)
    nc.sync.dma_start(out=whp_f[:], in_=whp_e.ap().rearrange("a k -> k a"))
    whp_b = cpool.tile([K, 1], BF16)
    nc.vector.tensor_copy(whp_b[:], whp_f[:])
    whr_f = cpool.tile([K, 1], F32)
    nc.sync.dma_start(out=whr_f[:], in_=whr_e.ap().rearrange("a k -> k a"))
    whr_b = cpool.tile([K, 1], BF16)
    nc.vector.tensor_copy(whr_b[:], whr_f[:])

    # Full-resident fp32 inputs (kept for the pooling phase):
    # R_all[p, b, i, d] = review[b, i*128+p, d];  P_all[p, b, j, d]
    r_all = inpool.tile([128, B_LOC, TI, D], F32)
    p_all = inpool.tile([128, B_LOC, NI, D], F32)

    # Per-batch logits collected as (128, 12) columns: cols 0:4 ap-tiles, 4:12 ar-tiles
    lgt_all = inpool.tile([128, 12, B_LOC], F32)

    # ---------------- per-batch main phase ----------------
    for b in range(B_LOC):
        # Load inputs
        nc.sync.dma_start(
            out=r_all[:, b], in_=rev_e.ap()[b].rearrange("(i p) d -> p i d", p=128)
        )
        nc.sync.dma_start(
            out=p_all[:, b], in_=post_e.ap()[b].rearrange("(j p) d -> p j d", p=128)
        )

        # bf16 casts
        r_bf = wk.tile([128, TI, D], BF16, tag="r_bf")
        nc.vector.tensor_copy(r_bf[:], r_all[:, b])
        p_bf = wk.tile([128, NI, D], BF16, tag="p_bf")
        nc.vector.tensor_copy(p_bf[:], p_all[:, b])

        # Transpose R -> Rt (D, T) into HrIn[0:64]; P -> Pt (D, N) into HpIn[0:64]
        hr_in = wk.tile([128, T], BF16, tag="hr_in")   # [Rt ; Y]
        hp_in = wk.tile([128, N], BF16, tag="hp_in")   # [Pt ; X]
        rt_ps = ps_tp.tile([D, T], BF16, tag="tp")
        for i in range(TI):
            nc.tensor.transpose(rt_ps[:, i * 128:(i + 1) * 128], r_bf[:, i], ident_b[:])
        nc.vector.tensor_copy(hr_in[0:D, :], rt_ps[:])
        pt_ps = ps_tp.tile([D, N], BF16, tag="tp")
        for j in range(NI):
            nc.tensor.transpose(pt_ps[:, j * 128:(j + 1) * 128], p_bf[:, j], ident_b[:])
        nc.vector.tensor_copy(hp_in[0:D, :], pt_ps[:])

        # RlT = Wl^T-contract:  RlT[d', t] = sum_d Wl[d, d'] * Rt[d, t]   (D, T)
        rlt = wk.tile([D, T], BF16, tag="rlt")
        for c in range(2):
            rlt_ps = ps_mm.tile([D, 512], F32, tag="mm")
            nc.tensor.matmul(
                rlt_ps[:],
                wl_b[:],
                hr_in[0:D, c * 512:(c + 1) * 512],
            )
            nc.vector.tensor_copy(rlt[:, c * 512:(c + 1) * 512], rlt_ps[:])

        # L tiles: L_i (128, N) = tanh(RlT[:, i-chunk]^T @ Pt)
        l_sb = wk.tile([128, TI, N], BF16, tag="l_sb")
        for i in range(TI):
            lps = ps_mm.tile([128, N], F32, tag="mm")
            nc.tensor.matmul(lps[:], rlt[:, i * 128:(i + 1) * 128], hp_in[0:D, :])
            nc.scalar.activation(l_sb[:, i], lps[:], AF.Tanh)

        # LT tiles via PE block-transposes of tanh'd L:
        # LT[:, j, i*128:(i+1)*128] = L_sb[:, i, j*128:(j+1)*128]^T
        lt_sb = wk.tile([128, NI, T], BF16, tag="lt_sb")
        for j in range(NI):
            ltp = ps_tp.tile([128, T], BF16, tag="tp")
            for i in range(TI):
                nc.tensor.transpose(
                    ltp[:, i * 128:(i + 1) * 128],
                    l_sb[:, i, j * 128:(j + 1) * 128],
                    ident_b[:],
                )
            nc.vector.tensor_copy(lt_sb[:, j], ltp[:])

        # X = R^T @ L  (D, N), accumulated over t-tiles -> HpIn[64:128]
        xps = ps_mm.tile([D, N], F32, tag="mm")
        for i in range(TI):
            nc.tensor.matmul(
                xps[:], r_bf[:, i], l_sb[:, i], start=(i == 0), stop=(i == TI - 1)
            )
        nc.scalar.copy(hp_in[D:128, :], xps[:])

        # Y = P^T @ L^T  (D, T), accumulated over n-tiles -> HrIn[64:128]
        for c in range(2):
            yps = ps_mm.tile([D, 512], F32, tag="mm")
            for j in range(NI):
                nc.tensor.matmul(
                    yps[:],
                    p_bf[:, j],
                    lt_sb[:, j, c * 512:(c + 1) * 512],
                    start=(j == 0),
                    stop=(j == NI - 1),
                )
            nc.scalar.copy(hr_in[D:128, c * 512:(c + 1) * 512], yps[:])

        # Hp = tanh(WT_hp^T @ HpIn)  (K, N);  Hr = tanh(WT_hr^T @ HrIn)  (K, T)
        hp_bf = wk.tile([K, N], BF16, tag="hp_bf")
        hps = ps_mm.tile([K, N], F32, tag="mm")
        nc.tensor.matmul(hps[:], wt_hp[:], hp_in[:])
        nc.scalar.activation(hp_bf[:], hps[:], AF.Tanh)
        hr_bf = wk.tile([K, T], BF16, tag="hr_bf")
        for c in range(2):
            hrs = ps_mm.tile([K, 512], F32, tag="mm")
            nc.tensor.matmul(hrs[:], wt_hr[:], hr_in[:, c * 512:(c + 1) * 512])
            nc.scalar.activation(hr_bf[:, c * 512:(c + 1) * 512], hrs[:], AF.Tanh)

        # logits, transposed orientation: (128, 1) per 128-chunk via thin matmuls
        lg_ps = ps_sm.tile([128, 12], F32, tag="sm")
        for j in range(NI):
            nc.tensor.matmul(
                lg_ps[:, j:j + 1],
                hp_bf[:, j * 128:(j + 1) * 128],
                whp_b[:],
                skip_group_check=True,
            )
        for i in range(TI):
            nc.tensor.matmul(
                lg_ps[:, 4 + i:5 + i],
                hr_bf[:, i * 128:(i + 1) * 128],
                whr_b[:],
                skip_group_check=True,
            )
        nc.vector.tensor_copy(lgt_all[:, :, b], lg_ps[:])

    # ---------------- softmax phase (all batches on partitions) ----------------
    # Transpose logits to (B_LOC, 1536): cols 0:512 ap-logits, 512:1536 ar-logits
    logits = inpool.tile([B_LOC, 12 * 128], F32)
    for g in range(3):
        lgt_t_ps = ps_sm.tile([B_LOC, 512], F32, tag="sm")
        for jj in range(4):
            j = g * 4 + jj
            nc.tensor.transpose(
                lgt_t_ps[:, jj * 128:(jj + 1) * 128], lgt_all[:, j, :], ident_f[:]
            )
        nc.vector.tensor_copy(logits[:, g * 512:(g + 1) * 512], lgt_t_ps[:])

    mx = inpool.tile([B_LOC, 2], F32)
    nc.vector.reduce_max(mx[:, 0:1], logits[:, 0:N], axis=mybir.AxisListType.X)
    nc.vector.reduce_max(mx[:, 1:2], logits[:, N:N + T], axis=mybir.AxisListType.X)
    nmx = inpool.tile([B_LOC, 2], F32)
    nc.vector.tensor_scalar_mul(nmx[:], mx[:], -1.0)

    probs = inpool.tile([B_LOC, 12 * 128], F32)
    sums = inpool.tile([B_LOC, 2], F32)
    nc.scalar.activation(
        probs[:, 0:N], logits[:, 0:N], AF.Exp, bias=nmx[:, 0:1], accum_out=sums[:, 0:1]
    )
    nc.scalar.activation(
        probs[:, N:N + T], logits[:, N:N + T], AF.Exp, bias=nmx[:, 1:2],
        accum_out=sums[:, 1:2],
    )
    rcp = inpool.tile([B_LOC, 2], F32)
    nc.vector.reciprocal(rcp[:], sums[:])
    pn = inpool.tile([B_LOC, 12 * 128], F32)
    nc.vector.tensor_scalar_mul(pn[:, 0:N], probs[:, 0:N], rcp[:, 0:1])
    nc.vector.tensor_scalar_mul(pn[:, N:N + T], probs[:, N:N + T], rcp[:, 1:2])

    # Transpose probs back to partition-major: PrT[:, j, b]
    prt_ps = ps_sm.tile([128, 12 * B_LOC], F32, tag="sm")
    for j in range(12):
        nc.tensor.transpose(
            prt_ps[:, j * B_LOC:(j + 1) * B_LOC],
            pn[:, j * 128:(j + 1) * 128],
            ident_f[0:B_LOC, 0:B_LOC],
        )
    prt = inpool.tile([128, 12, B_LOC], F32)
    nc.vector.tensor_copy(prt[:], prt_ps[:])

    # ---------------- pooling phase ----------------
    # co_all (64, 16): col b = P_b^T @ Ap_b, col 8+b = R_b^T @ Ar_b
    co_ps = ps_sm.tile([D, 2 * B_LOC], F32, tag="sm")
    for b in range(B_LOC):
        for j in range(NI):
            nc.tensor.matmul(
                co_ps[:, b:b + 1], p_all[:, b, j], prt[:, j, b:b + 1],
                start=(j == 0), stop=(j == NI - 1), skip_group_check=True,
            )
        for i in range(TI):
            nc.tensor.matmul(
                co_ps[:, B_LOC + b:B_LOC + b + 1], r_all[:, b, i], prt[:, 4 + i, b:b + 1],
                start=(i == 0), stop=(i == TI - 1), skip_group_check=True,
            )
    co_sb = inpool.tile([D, 2 * B_LOC], F32)
    nc.vector.tensor_copy(co_sb[:], co_ps[:])

    # Transpose (64, 16) -> (16, 64); row h*8+b is the h-half of out[b]
    cot_ps = ps_sm.tile([2 * B_LOC, D], F32, tag="sm")
    nc.tensor.transpose(cot_ps[:], co_sb[:], ident_f[0:D, 0:D])
    out_sb = inpool.tile([2 * B_LOC, D], F32)
    nc.vector.tensor_copy(out_sb[:], cot_ps[:])
    nc.sync.dma_start(out=out_e.ap()[:, 0:D], in_=out_sb[0:B_LOC, :])
    nc.sync.dma_start(out=out_e.ap()[:, D:2 * D], in_=out_sb[B_LOC:2 * B_LOC, :])
    ctx.close()


_NC_CACHE = None


def _get_nc():
    global _NC_CACHE
    if _NC_CACHE is None:
        _NC_CACHE = build_kernel()
    return _NC_CACHE


def run_on_hw(inputs: dict, trace: bool = False, **kw):
    nc = _get_nc()
    rev = np.ascontiguousarray(np.asarray(inputs["review_seq"], dtype=np.float32))
    post = np.ascontiguousarray(np.asarray(inputs["post_seq"], dtype=np.float32))
    weights = {
        k: np.ascontiguousarray(np.asarray(inputs[k], dtype=np.float32))
        for k in ("Wl", "Wr", "Wp", "whr", "whp")
    }
    ident = np.eye(128, dtype=np.float32)
    in_maps = []
    for c in range(NCORES):
        m = {
            "review_seq": np.ascontiguousarray(rev[c * B_LOC:(c + 1) * B_LOC]),
            "post_seq": np.ascontiguousarray(post[c * B_LOC:(c + 1) * B_LOC]),
            "ident": ident,
        }
        m.update(weights)
        in_maps.append(m)
    res = run_bass_kernel_spmd(nc, in_maps, list(range(NCORES)), trace=trace, **kw)
    out = np.concatenate([res.results[c]["out"] for c in range(NCORES)], axis=0)
    return out, res


def kernel(**inputs) -> np.ndarray:
    out, _ = run_on_hw(inputs, trace=False)
    return out.astype(np.float32)
